# revision 42
# baseline (speedup 1.0000x reference)
"""ArcFace loss (margin=0.3, scale=30) on 8 TRN2 NeuronCores.

Vocab-parallel sharding: weight rows (classes) are split across the 8
cores. Per core, on device: weight-row norms via an f32r ones-matmul
sumsq (+ Ln/Exp rsqrt on ScalarE), w^T tiles scaled by SCALE/||w|| on
VectorE, the [1024 x 12800] logit block via f32r TensorE matmuls, and
sum(exp(logit - shift)) per 512-class tile via ScalarE Exp+accum. The
shift is a host-computed per-row safe bound (6*SCALE*||e_b||/sqrt(D)),
so no on-device max pass is needed. The host sums the 200 partial sums
per row in fp64 and applies the ArcFace target-column phi substitution
as a rank-1 correction (O(B*D) work).

kernel(**inputs) takes the FULL unsharded inputs and returns the FULL
(scalar) output, matching the reference.
"""

import math
import os
import sys

import numpy as np

for _p in ("/opt/trn_rl_repo", "/root/.axon_site/_ro/trn_rl_repo"):
    if os.path.isdir(_p) and _p not in sys.path:
        sys.path.insert(0, _p)

from concourse import bacc, bass, tile  # noqa: E402
from concourse import mybir  # noqa: E402
from concourse.bass_utils import run_bass_kernel_spmd  # noqa: E402
from concourse.tile import add_dep_helper  # noqa: E402

MARGIN = 0.3
SCALE = 30.0
NUM_CLASSES = 100000
EMB_DIM = 512
BATCH = 1024
N_CORES = 8

CS = 12800  # padded classes per core shard (8*12800 = 102400 >= 100000)
CT = 512  # classes per tile (one psum bank of fp32)
NCT = CS // CT  # 25 class tiles per core
ND = EMB_DIM // 128  # 4 contraction tiles
NB = BATCH // 128  # 8 batch tiles
CHUNK = 5  # c-tiles per software-pipeline chunk (groups ACT funcs)
MM_DTYPE = os.environ.get("MM_DTYPE", "f32r")  # f32 | f32r | bf16
MODE = os.environ.get("MODE", "bias")  # bias (host-computed safe shift) | exact
BIAS_SIGMA = 6.0  # shift = BIAS_SIGMA * SCALE * ||e_b|| / sqrt(D)
REPEAT = int(os.environ.get("REPEAT", "1"))  # bench amplification loop
STRIP = os.environ.get("STRIP", "")  # "" | noepi (bench-only variants)
F32 = mybir.dt.float32


def build_nc():
    """Per-core SPMD graph. Inputs: emb_t [512,1024] (full embeddings,
    transposed), wt [512,CS] (transposed weight shard), nbias [128,NB]
    (bias mode: negated per-row safe exp shift). Output (bias mode):
    out [128, NB*NCT] = sum(exp(logit - shift)) per (b-tile, c-tile).
    In exact mode out is [128, 2*NB*NCT]: per-tile -max then sums."""
    nc = bacc.Bacc(None, target_bir_lowering=False, debug=False)
    emb_t = nc.declare_dram_parameter("emb_t", [EMB_DIM, BATCH], F32, isOutput=False)
    wt = nc.declare_dram_parameter("wt", [EMB_DIM, CS], F32, isOutput=False)
    if MODE == "bias":
        nbias = nc.declare_dram_parameter("nbias", [128, NB], F32, isOutput=False)
        out = nc.declare_dram_parameter("out", [128, NB * NCT], F32, isOutput=True)
    else:
        out = nc.declare_dram_parameter("out", [128, 2 * NB * NCT], F32, isOutput=True)

    AF = mybir.ActivationFunctionType
    with tile.TileContext(nc) as tc:
        with (
            tc.tile_pool(name="dram", bufs=1, space="DRAM") as dpool,
            tc.tile_pool(name="emb", bufs=1) as epool,
            tc.tile_pool(name="stats", bufs=1) as spool,
            tc.tile_pool(name="norm", bufs=2) as npool,
            tc.tile_pool(name="small", bufs=CHUNK + 2) as smpool,
            tc.tile_pool(name="wtile", bufs=CHUNK) as wtpool,
            tc.tile_pool(name="wnorm", bufs=2 * CHUNK + 1) as wnpool,
            tc.tile_pool(name="junk", bufs=2) as jpool,
            tc.tile_pool(name="psum", bufs=6, space="PSUM") as ppool,
            tc.tile_pool(name="psum1", bufs=2, space="PSUM") as p1pool,
        ):
            t_sb = spool.tile([128, NB * NCT], F32, tag="t_sb")
            m_sb = None
            nb_sb = None
            if MODE == "bias":
                nb_sb = spool.tile([128, NB], F32, tag="nb_sb")
                nc.sync.dma_start(out=nb_sb[:], in_=nbias[:])
            else:
                m_sb = spool.tile([128, NB * NCT], F32, tag="m_sb")
            eps = spool.tile([128, 1], F32, tag="eps")
            nc.gpsimd.memset(eps[:], 1e-12)
            if STRIP:
                nc.gpsimd.memset(t_sb[:], 0.0)
                if m_sb is not None:
                    nc.gpsimd.memset(m_sb[:], 0.0)

            mm_dt = {
                "f32": F32,
                "f32r": mybir.dt.float32r,
                "bf16": mybir.dt.bfloat16,
            }[MM_DTYPE]
            # embeddings^T resident in SBUF: 4 tiles [128d, 1024b]. For
            # f32r matmul the operand must be produced as f32r (rounded),
            # so cast through a DVE copy.
            e_sb = []
            for d in range(ND):
                et = epool.tile([128, BATCH], mm_dt, tag=f"e{d}")
                if MM_DTYPE in ("f32r", "bf16"):
                    etmp = npool.tile([128, BATCH], F32, tag="etmp")
                    nc.sync.dma_start(
                        out=etmp[:], in_=emb_t[d * 128 : (d + 1) * 128, :]
                    )
                    nc.vector.tensor_copy(et[:], etmp[:])
                else:
                    nc.sync.dma_start(out=et[:], in_=emb_t[d * 128 : (d + 1) * 128, :])
                e_sb.append(et)

            # wt rows grouped by d-chunk: [128, 4, CS]
            wt_v = wt.rearrange("(dc p) c -> p dc c", p=128)

            # ones column for the sumsq matmul (stationary [128, 1]);
            # memset can't write f32r, so round through a DVE copy
            ones_f = spool.tile([128, 1], F32, tag="ones_f")
            nc.gpsimd.memset(ones_f[:], 1.0)
            ones = spool.tile([128, 1], mybir.dt.float32r, tag="ones")
            nc.vector.tensor_copy(ones[:], ones_f[:])

            def produce(chunk):
                """Build scaled w^T tiles for the chunk's c-tiles. Row norms
                come from the same wt tiles: square on ACT (f32r), sumsq
                over d via a ones-matmul into [1, CT] PSUM, then
                r = exp(-ln(ss)/2) on one partition, broadcast by GpSimd.
                ACT ops grouped by function to avoid act-table reloads."""
                cts = range(chunk * CHUNK, (chunk + 1) * CHUNK)
                pss = {}
                wtws = {}
                wns = {}
                for ct in cts:
                    wtw = wtpool.tile([128, ND, CT], F32, tag="wtw")
                    wtws[ct] = wtw
                    nc.sync.dma_start(
                        out=wtw[:], in_=wt_v[:, :, ct * CT : (ct + 1) * CT]
                    )
                    # squares staged in the future wn slot (overwritten by
                    # the scaled weights after the sumsq matmul reads them);
                    # on DVE to keep ACT free for the main exp stream
                    wnw = wnpool.tile([128, ND, CT], mm_dt, tag="wn")
                    wns[ct] = wnw
                    for d in range(ND):
                        nc.vector.tensor_mul(wnw[:, d, :], wtw[:, d, :], wtw[:, d, :])
                    ps1 = p1pool.tile([1, CT], F32, tag="ps1")
                    for d in range(ND):
                        nc.tensor.matmul(
                            ps1[:],
                            ones[:],
                            wnw[:, d, :],
                            start=(d == 0),
                            stop=(d == ND - 1),
                        )
                    pss[ct] = ps1
                ln_insts = []
                act_insts = []
                lns = {}
                for ct in cts:
                    # l = ln(ss/SCALE^2 + eps); r = exp(-l/2) = SCALE/||w||.
                    # Ln+Exp live in one ACT table set (unlike Sqrt) -> no
                    # act-table reloads against the main-pass Exp stream.
                    lnr = smpool.tile([1, CT], F32, tag="lnr")
                    ln_insts.append(
                        nc.scalar.activation(
                            lnr[:],
                            pss[ct][:],
                            AF.Ln,
                            scale=1.0 / (SCALE * SCALE),
                            bias=eps[:1, :],
                        )
                    )
                    lns[ct] = lnr
                rrows = {}
                for ct in cts:
                    rr = smpool.tile([1, CT], F32, tag="rr")
                    act_insts.append(
                        nc.scalar.activation(rr[:], lns[ct][:], AF.Exp, scale=-0.5)
                    )
                    rrows[ct] = rr
                # keep all Lns before all Exps in the ACT stream (one table
                # switch per block instead of one per ct)
                add_dep_helper(
                    act_insts[0].ins,
                    ln_insts[-1].ins,
                    sync=False,
                    reason="group ACT: exp(-l/2) block after ln block",
                )
                act_insts = ln_insts + act_insts
                wn = {}
                for ct in cts:
                    rb = npool.tile([128, CT], F32, tag="rb")
                    nc.gpsimd.partition_broadcast(rb[:], rrows[ct][:])
                    wnw = wns[ct]
                    for d in range(ND):
                        nc.vector.tensor_mul(
                            wnw[:, d, :], wtws[ct][:, d, :], rb[:]
                        )
                    wn[ct] = wnw
                return wn, act_insts

            def consume(chunk, wn):
                """Matmul + per-tile LSE stats for the chunk's c-tiles."""
                exp_insts = []
                for ct in range(chunk * CHUNK, (chunk + 1) * CHUNK):
                    for bt in range(NB):
                        ps = ppool.tile([128, CT], F32, tag="ps")
                        wct = ct % CHUNK if STRIP in ("justmm", "mmonly") else ct
                        for d in range(ND):
                            nc.tensor.matmul(
                                ps[:],
                                e_sb[d][:, bt * 128 : (bt + 1) * 128],
                                wn[wct][:, d, :],
                                start=(d == 0),
                                stop=(d == ND - 1),
                            )
                        if STRIP in ("noepi", "mmonly"):
                            continue
                        col = bt * NCT + ct
                        if MODE == "bias":
                            bias_ap = nb_sb[:, bt : bt + 1]
                        else:
                            nc.vector.reduce_max(
                                m_sb[:, col : col + 1],
                                ps[:],
                                axis=mybir.AxisListType.X,
                                negate=True,
                            )
                            bias_ap = m_sb[:, col : col + 1]
                        junk = jpool.tile([128, CT], F32, tag="junk")
                        exp_insts.append(
                            nc.scalar.activation(
                                junk[:],
                                ps[:],
                                AF.Exp,
                                bias=bias_ap,
                                accum_out=t_sb[:, col : col + 1],
                            )
                        )
                return exp_insts

            # Software pipeline: produce chunk c+1's normalized w^T tiles
            # while the PE consumes chunk c. Order-only deps group the ACT
            # stream as [prepass(c+1)][main-exp(c)] so the act-table stays
            # on one function set per block.
            def pipeline_body():
                n_chunks = NCT // CHUNK
                if STRIP in ("justmm", "mmonly"):
                    wn_cur, _ = produce(0)
                    for c in range(n_chunks):
                        consume(c, wn_cur)
                    return
                wn_cur, _ = produce(0)
                prev_exps = None
                for c in range(n_chunks):
                    wn_next, pre_next = (None, None)
                    if c + 1 < n_chunks:
                        wn_next, pre_next = produce(c + 1)
                        if prev_exps:
                            add_dep_helper(
                                pre_next[0].ins,
                                prev_exps[-1].ins,
                                sync=False,
                                reason="group ACT: prepass after prior exp block",
                            )
                    exps = consume(c, wn_cur)
                    if pre_next is not None and exps:
                        add_dep_helper(
                            exps[0].ins,
                            pre_next[-1].ins,
                            sync=False,
                            reason="group ACT: exp block after next prepass block",
                        )
                    wn_cur, prev_exps = wn_next, exps

            if REPEAT > 1:
                with tc.For_i(0, REPEAT):
                    pipeline_body()
            else:
                pipeline_body()

            if MODE == "bias":
                nc.sync.dma_start(out=out[:], in_=t_sb[:])
            else:
                nc.sync.dma_start(out=out[:, : NB * NCT], in_=m_sb[:])
                nc.sync.dma_start(out=out[:, NB * NCT :], in_=t_sb[:])
    nc.compile()
    return nc


def neg_bias(embeddings):
    """-shift per batch row: shift = BIAS_SIGMA*SCALE*||e_b||/sqrt(D).
    Safely above the max logit (overflow would need an ~8.8 sigma cosine)
    while keeping exp sums well inside fp32 range. Layout [128, NB]."""
    e = np.asarray(embeddings, dtype=np.float64)
    shift = BIAS_SIGMA * SCALE * np.sqrt((e * e).sum(1)) / np.sqrt(EMB_DIM)
    return -shift.astype(np.float32).reshape(NB, 128).T.copy(), shift


def make_in_maps(embeddings, weight):
    emb_t = np.ascontiguousarray(embeddings.T.astype(np.float32, copy=False))
    w_pad = np.zeros((N_CORES * CS, EMB_DIM), dtype=np.float32)
    w_pad[:NUM_CLASSES] = weight
    nb = neg_bias(embeddings)[0] if MODE == "bias" else None
    in_maps = []
    for i in range(N_CORES):
        shard = w_pad[i * CS : (i + 1) * CS]
        m = {
            "emb_t": emb_t,
            "wt": np.ascontiguousarray(shard.T),
        }
        if nb is not None:
            m["nbias"] = nb
        in_maps.append(m)
    return in_maps


def combine(outs, embeddings, labels, weight):
    """Host-side fp64 combine of per-core partial LSE stats + ArcFace
    target-column correction. outs: list of 8 arrays [128, 400]."""
    cos_m = math.cos(MARGIN)
    sin_m = math.sin(MARGIN)
    th = math.cos(math.pi - MARGIN)
    mm = math.sin(math.pi - MARGIN) * MARGIN

    if MODE == "bias":
        M = neg_bias(embeddings)[1]  # [1024] the known shift
        S = np.zeros(BATCH)
        for o in outs:
            t = np.asarray(o, dtype=np.float64).reshape(128, NB, NCT)
            S += np.transpose(t, (1, 0, 2)).reshape(BATCH, NCT).sum(axis=1)
    else:
        m_parts = []  # each [1024, NCT] logit-unit maxes
        t_parts = []
        for o in outs:
            o = np.asarray(o, dtype=np.float64)
            neg_m = o[:, : NB * NCT].reshape(128, NB, NCT)
            t = o[:, NB * NCT :].reshape(128, NB, NCT)
            # batch index b = bt*128 + p
            m_parts.append(-np.transpose(neg_m, (1, 0, 2)).reshape(BATCH, NCT))
            t_parts.append(np.transpose(t, (1, 0, 2)).reshape(BATCH, NCT))
        m_all = np.concatenate(m_parts, axis=1)  # [1024, 200]
        t_all = np.concatenate(t_parts, axis=1)

        M = m_all.max(axis=1)  # [1024] global logit max (>= target logit)
        S = np.sum(t_all * np.exp(m_all - M[:, None]), axis=1)  # [1024]

    # target-column correction: replace exp(s*cos_t) by exp(s*phi_t)
    lab = np.asarray(labels).astype(np.int64)
    e64 = np.asarray(embeddings, dtype=np.float64)
    w_tgt = np.asarray(weight, dtype=np.float64)[lab]  # [1024, 512]
    nrm_t = np.sqrt(np.sum(w_tgt * w_tgt, axis=1))
    cos_t = np.sum(e64 * w_tgt, axis=1) / nrm_t
    sine = np.sqrt(np.clip(1.0 - cos_t * cos_t, 0.0, 1.0))
    phi = cos_t * cos_m - sine * sin_m
    phi = np.where(cos_t > th, phi, cos_t - mm)

    S_corr = S - np.exp(SCALE * cos_t - M) + np.exp(SCALE * phi - M)
    nll = np.log(S_corr) + M - SCALE * phi
    return np.float32(np.mean(nll))


def kernel(embeddings, labels, weight):
    nc = build_nc()
    in_maps = make_in_maps(embeddings, weight)
    res = run_bass_kernel_spmd(nc, in_maps, core_ids=list(range(N_CORES)))
    outs = [res.results[i]["out"] for i in range(N_CORES)]
    return combine(outs, embeddings, labels, weight)
